# revision 14
# baseline (speedup 1.0000x reference)
"""Distributed ContrastiveMoCoKnnBert loss kernel for 8 trn2 NeuronCores.

Math reduction (exact, not approximate):
  loss_con = -mean(log_softmax([pos | negs] / T)[:, 0]) over (B*TOP_K) rows.
  For row (b, j):  term = log(exp(p_bj/T) + sum_neg exp(n/T)) - p_bj/T
  where p_bj = j-th largest of cos_sim[b, :] (over ALL K columns) and the
  negative sum runs over columns whose queue label != labels[b].  The
  reference's top-NEG_MIN sort is irrelevant: softmax denominators are
  permutation invariant.  So the kernel only needs, per batch row:
    * top-25 values of cos_sim[b, :]        (monotonic under exp -> we
      extract top exp-values instead)
    * S_all[b] = sum_k exp(cos/T), S_pos[b] = sum_{label match} exp(cos/T)

Sharding: feature_queue is sorted by label on the host (1024 rows per
label, exactly balanced by construction), transposed, tiled, and split
along K into 8 shards of 8192 (= 8 labels x 1024) -- one per core:
  1. linerT = (tanh(q@Wd+bd)@Wo+bo).T unnormalized via transpose-free
     matmuls (host supplies qT in partition-major layout), plus its
     column norms (ones-matmul over squared entries)
  2. stream the fp8(e4m3, x256 host scale) fqT shard through TensorE:
     cos chunks [128, 512] f32 psum (partition = batch b + 64*half)
  3. ScalarE Exp with per-partition scale (2/(256*norm_b)) fused with
     per-512-chunk accum sums -> acc[128, 8]  (label-group sums = pairs
     of chunks, since each label spans 1024 sorted columns)
  4. one VectorE MAX8 per 512-col chunk -> top-8 bucket candidates
     cand[128, 64]; host PROVES completeness (bucket 8th-largest <=
     global 25th candidate) -- holds by a huge margin for random data
  5. classification head logits (replicated) -> lcT[63, 64]
Host merges: top-25 of the 1024 per-row candidates, S_neg = S_all-S_pos,
and assembles the scalar loss in f64.  All O(B*K*H) work is on device.
"""

import os

import numpy as np

import concourse.bass as bass
import concourse.bacc as bacc
import concourse.tile as tile
from concourse import mybir
from concourse.bass_utils import run_bass_kernel_spmd

B = 64
H = 768
K = 65536
L = 64            # NUM_LABELS
TOP_K = 25
T = 0.5
NCORES = 8
KSH = K // NCORES         # 8192 queue rows per core
NKC = H // 128            # 6 contraction chunks
CHUNK = 512               # psum-bank sized cos chunk
NJ = KSH // 2 // CHUNK    # 8 chunks per half
NH = 2                    # halves (partition packing: p = b + 64*h)
NCAND = 8                 # top-8 extracted per 512-col chunk

F32 = mybir.dt.float32
BF16 = mybir.dt.bfloat16
FP8 = mybir.dt.float8e4
FQ_FP8 = True             # stream feature queue as fp8e4m3 (x256 host scale)
FQ_DT = FP8 if FQ_FP8 else BF16
FQ_SCALE = 256.0 if FQ_FP8 else 1.0

_cache: dict = {}

last_exec_time_ns: int | None = None
last_results = None


def _ensure_ntff_hook():
    """Register the axon NTFF profiling hook if the image's antenv lacks
    the ``axon_hooks`` module (the hook impl itself ships in
    trn_agent_boot).  Also keep trace artifacts local instead of
    uploading to a share bucket."""
    import sys
    import types

    import concourse.bass_utils as bu

    bu.upload_artifacts = lambda tmpdir: tmpdir
    try:
        from antenv.axon_hooks import get_axon_ntff_profile_hook  # noqa: F401
        return
    except ImportError:
        pass
    try:
        from trn_agent_boot.trn_boot import _ntff_profile_via_ctypes
    except ImportError:
        return
    mod = types.ModuleType("antenv.axon_hooks")
    _hook = [None]
    mod.set_axon_ntff_profile_hook = lambda h: _hook.__setitem__(0, h)
    mod.get_axon_ntff_profile_hook = lambda: _hook[0]
    sys.modules["antenv.axon_hooks"] = mod
    import antenv

    antenv.axon_hooks = mod
    try:
        mod.set_axon_ntff_profile_hook(
            _ntff_profile_via_ctypes("/opt/axon/libaxon_pjrt.so")
        )
    except Exception:
        mod.set_axon_ntff_profile_hook(None)


def _build_nc():
    nc = bacc.Bacc(
        "TRN2",
        target_bir_lowering=False,
        debug=False,
        enable_asserts=False,
        num_devices=NCORES,
    )

    qT = nc.dram_tensor("qT", [128, NKC, B], BF16, kind="ExternalInput")
    wd = nc.dram_tensor("wd", [128, NKC, H], BF16, kind="ExternalInput")
    wo = nc.dram_tensor("wo", [128, NKC, H], BF16, kind="ExternalInput")
    wc1 = nc.dram_tensor("wc1", [128, NKC, H], BF16, kind="ExternalInput")
    wc2 = nc.dram_tensor("wc2", [128, NKC, L - 1], BF16, kind="ExternalInput")
    bdt = nc.dram_tensor("bd", [128, NKC], F32, kind="ExternalInput")
    bot = nc.dram_tensor("bo", [128, NKC], F32, kind="ExternalInput")
    bc1t = nc.dram_tensor("bc1", [128, NKC], F32, kind="ExternalInput")
    bc2t = nc.dram_tensor("bc2", [L - 1, 1], F32, kind="ExternalInput")
    fqt = nc.dram_tensor("fqt", [NJ, NKC, 128, NH * CHUNK], FQ_DT, kind="ExternalInput")

    cand_o = nc.dram_tensor("cand", [128, NJ * NCAND], BF16, kind="ExternalOutput")
    acc_o = nc.dram_tensor("acc", [128, NJ], F32, kind="ExternalOutput")
    lc_o = nc.dram_tensor("lcT", [L - 1, B], F32, kind="ExternalOutput")

    AF = mybir.ActivationFunctionType

    with tile.TileContext(nc) as tc:
        with (
            tc.tile_pool(name="weights", bufs=1) as wpool,
            tc.tile_pool(name="work", bufs=1) as spool,
            tc.tile_pool(name="fqstream", bufs=36) as fqpool,
            tc.tile_pool(name="cospsum", bufs=6, space="PSUM") as pspool,
            tc.tile_pool(name="headpsum", bufs=2, space="PSUM") as hpool,
        ):
            # ---- resident params --------------------------------------
            wd_sb = wpool.tile([128, NKC, H], BF16)
            wo_sb = wpool.tile([128, NKC, H], BF16)
            wc1_sb = wpool.tile([128, NKC, H], BF16)
            wc2_sb = wpool.tile([128, NKC, L - 1], BF16)
            qt_sb = wpool.tile([128, NKC, B], BF16)
            bd_sb = wpool.tile([128, NKC], F32)
            bo_sb = wpool.tile([128, NKC], F32)
            bc1_sb = wpool.tile([128, NKC], F32)
            bc2_sb = wpool.tile([L - 1, 1], F32)
            ones_sb = wpool.tile([128, 1], F32)

            # critical path (contrastive head) on the HWDGE queue, ahead
            # of the fq stream; cls-head params via SWDGE in parallel
            nc.sync.dma_start(wd_sb[:], wd.ap())
            nc.sync.dma_start(qt_sb[:], qT.ap())
            nc.sync.dma_start(bd_sb[:], bdt.ap())
            nc.sync.dma_start(bo_sb[:], bot.ap())
            nc.sync.dma_start(wo_sb[:], wo.ap())
            nc.gpsimd.dma_start(wc1_sb[:], wc1.ap())
            nc.gpsimd.dma_start(wc2_sb[:], wc2.ap())
            nc.gpsimd.dma_start(bc1_sb[:], bc1t.ap())
            nc.gpsimd.dma_start(bc2_sb[:], bc2t.ap())
            nc.vector.memset(ones_sb[:], 1.0)

            # ---- contrastive head: linerT (unnormalized) + norms ------
            h1_sb = spool.tile([128, NKC, B], BF16)
            pre2_sb = spool.tile([128, NKC, B], F32)
            pre2b_sb = spool.tile([128, NKC, B], BF16)
            sq_sb = spool.tile([128, NKC, B], F32)

            for mc in range(NKC):
                ps = hpool.tile([128, B], F32, tag="headps")
                for kc in range(NKC):
                    nc.tensor.matmul(
                        ps[:],
                        wd_sb[:, kc, mc * 128:(mc + 1) * 128],
                        qt_sb[:, kc, :],
                        start=(kc == 0),
                        stop=(kc == NKC - 1),
                    )
                nc.scalar.activation(
                    h1_sb[:, mc, :], ps[:], AF.Tanh, bias=bd_sb[:, mc:mc + 1]
                )

            for mc in range(NKC):
                ps = hpool.tile([128, B], F32, tag="headps")
                for kc in range(NKC):
                    nc.tensor.matmul(
                        ps[:],
                        wo_sb[:, kc, mc * 128:(mc + 1) * 128],
                        h1_sb[:, kc, :],
                        start=(kc == 0),
                        stop=(kc == NKC - 1),
                    )
                nc.vector.tensor_scalar_add(pre2_sb[:, mc, :], ps[:], bo_sb[:, mc:mc + 1])
                nc.vector.tensor_copy(pre2b_sb[:, mc, :], pre2_sb[:, mc, :])
                nc.vector.tensor_mul(sq_sb[:, mc, :], pre2_sb[:, mc, :], pre2_sb[:, mc, :])

            # column norms of pre2 (= row norms of liner_q), replicated to
            # both partition halves: norms2[p] for p = b and p = b + 64
            ps_n = hpool.tile([128, 1], F32, tag="headps")
            for hh in range(NH):
                for kc in range(NKC):
                    nc.tensor.matmul(
                        ps_n[hh * 64:(hh + 1) * 64, :],
                        sq_sb[:, kc, :],
                        ones_sb[:],
                        start=(kc == 0),
                        stop=(kc == NKC - 1),
                    )
            norm_sb = spool.tile([128, 1], F32)
            rcp_sb = spool.tile([128, 1], F32)
            scol_sb = spool.tile([128, 1], F32)
            nc.scalar.activation(norm_sb[:], ps_n[:], AF.Sqrt)
            nc.vector.reciprocal(rcp_sb[:], norm_sb[:])
            nc.vector.tensor_scalar_mul(scol_sb[:], rcp_sb[:], 1.0 / (T * FQ_SCALE))

            # ---- classification head (replicated, tiny) ---------------
            # emitted between head and stream: PE runs it while the cos
            # stream waits on fq DMA, keeping it off the critical tail
            h1c_sb = spool.tile([128, NKC, B], BF16)
            for mc in range(NKC):
                ps = hpool.tile([128, B], F32, tag="headps")
                for kc in range(NKC):
                    nc.tensor.matmul(
                        ps[:],
                        wc1_sb[:, kc, mc * 128:(mc + 1) * 128],
                        qt_sb[:, kc, :],
                        start=(kc == 0),
                        stop=(kc == NKC - 1),
                    )
                nc.scalar.activation(
                    h1c_sb[:, mc, :], ps[:], AF.Tanh, bias=bc1_sb[:, mc:mc + 1]
                )
            ps_l = hpool.tile([L - 1, B], F32, tag="headps")
            for kc in range(NKC):
                nc.tensor.matmul(
                    ps_l[:],
                    wc2_sb[:, kc, :],
                    h1c_sb[:, kc, :],
                    start=(kc == 0),
                    stop=(kc == NKC - 1),
                )
            lc_sb = spool.tile([L - 1, B], F32)
            nc.vector.tensor_scalar_add(lc_sb[:], ps_l[:], bc2_sb[:])
            nc.sync.dma_start(lc_o.ap(), lc_sb[:])

            # ---- main stream: cos chunks -> exp(+sums) -> topk --------
            acc_sb = spool.tile([128, NJ], F32)
            cand_sb = spool.tile([128, NJ * NCAND], BF16)

            for j in range(NJ):
                ps_c = pspool.tile([128, CHUNK], F32, tag="cos")
                fts = []
                for kc in range(NKC):
                    ft = fqpool.tile([128, NH * CHUNK], FQ_DT, tag="fq")
                    nc.sync.dma_start(ft[:], fqt.ap()[j, kc])
                    fts.append(ft)
                for hh in range(NH):
                    for kc in range(NKC):
                        nc.tensor.matmul(
                            ps_c[hh * 64:(hh + 1) * 64, :],
                            pre2b_sb[:, kc, :],
                            fts[kc][:, hh * CHUNK:(hh + 1) * CHUNK],
                            start=(kc == 0),
                            stop=(kc == NKC - 1),
                        )
                exp_t = fqpool.tile([128, CHUNK], BF16, tag="exp")
                nc.scalar.activation(
                    exp_t[:],
                    ps_c[:],
                    AF.Exp,
                    scale=scol_sb[:],
                    accum_out=acc_sb[:, j:j + 1],
                )
                nc.vector.max(cand_sb[:, j * NCAND:(j + 1) * NCAND], exp_t[:])

            nc.sync.dma_start(cand_o.ap(), cand_sb[:])
            nc.sync.dma_start(acc_o.ap(), acc_sb[:])

    nc.compile()
    return nc


def _get_nc():
    if "nc" not in _cache:
        _cache["nc"] = _build_nc()
    return _cache["nc"]


def _prep_inputs(q, label_queue, feature_queue, Wd, bd, Wo, bo, Wc1, bc1, Wc2, bc2):
    """Host-side shard/layout prep.  Returns per-core input maps."""
    lq = np.asarray(label_queue).astype(np.int64)
    counts = np.bincount(lq, minlength=L)
    assert counts.shape[0] == L and np.all(counts == K // L), (
        "kernel assumes an exactly balanced label queue"
    )
    perm = np.argsort(lq, kind="stable")
    fq_sorted = np.asarray(feature_queue, dtype=np.float32)[perm]  # [K, H]

    bf16 = mybir.dt.np(BF16)

    def pk(w, cols):  # [H, cols] -> partition-major [128, NKC, cols]
        return np.ascontiguousarray(
            np.asarray(w, np.float32).reshape(NKC, 128, cols).transpose(1, 0, 2)
        ).astype(bf16)

    common = {
        "qT": pk(np.asarray(q, np.float32).T, B),
        "wd": pk(Wd, H),
        "wo": pk(Wo, H),
        "wc1": pk(Wc1, H),
        "wc2": pk(Wc2, L - 1),
        "bd": np.ascontiguousarray(np.asarray(bd, np.float32).reshape(NKC, 128).T),
        "bo": np.ascontiguousarray(np.asarray(bo, np.float32).reshape(NKC, 128).T),
        "bc1": np.ascontiguousarray(np.asarray(bc1, np.float32).reshape(NKC, 128).T),
        "bc2": np.ascontiguousarray(np.asarray(bc2, np.float32).reshape(L - 1, 1)),
    }
    fq_dt = mybir.dt.np(FQ_DT)
    in_maps = []
    for c in range(NCORES):
        shard = fq_sorted[c * KSH:(c + 1) * KSH]          # [8192, H]
        fqT = np.ascontiguousarray(shard.T)               # [H, 8192]
        tiles = np.ascontiguousarray(
            (fqT * FQ_SCALE).reshape(NKC, 128, NH, NJ, CHUNK)
            .transpose(3, 0, 1, 2, 4)
            .reshape(NJ, NKC, 128, NH * CHUNK).astype(fq_dt)
        )
        in_maps.append({**common, "fqt": tiles})
    return in_maps


def kernel(
    q,
    labels,
    label_queue,
    feature_queue,
    Wd,
    bd,
    Wo,
    bo,
    Wc1,
    bc1,
    Wc2,
    bc2,
):
    global last_exec_time_ns, last_results
    nc = _get_nc()
    in_maps = _prep_inputs(
        q, label_queue, feature_queue, Wd, bd, Wo, bo, Wc1, bc1, Wc2, bc2
    )

    trace = os.environ.get("BASS_KERNEL_TRACE", "0") == "1"
    if trace:
        _ensure_ntff_hook()
    try:
        res = run_bass_kernel_spmd(
            nc,
            in_maps,
            core_ids=list(range(NCORES)),
            trace=trace,
            trace_cores=[0] if trace else None,
        )
    except Exception:
        if not trace:
            raise
        res = run_bass_kernel_spmd(nc, in_maps, core_ids=list(range(NCORES)))
    last_exec_time_ns = res.exec_time_ns
    last_results = res

    labels_np = np.asarray(labels).astype(np.int64)

    # ---- tiny host-side merge (the "gather + reduce" step) -----------
    C = np.stack([np.asarray(r["cand"]) for r in res.results]).astype(np.float64)
    A = np.stack([np.asarray(r["acc"]) for r in res.results]).astype(np.float64)

    # per-row candidate pool: cores x halves x (8 chunks * top-8)
    cand = np.concatenate([C[:, :B, :], C[:, B:, :]], axis=2)  # [8, 64, 128]
    cand = cand.transpose(1, 0, 2).reshape(B, -1)              # [64, 1024]
    e_top = np.sort(cand, axis=1)[:, ::-1][:, :TOP_K]          # exp(p/T) desc
    # Exactness proof: every unextracted value in a 512-col bucket is
    # <= that bucket's 8th-largest (MAX8 output is sorted desc).  If all
    # bucket minima are <= the global 25th candidate, the top-25 value
    # set is provably complete.
    bucket_min = np.concatenate(
        [C[:, :B, 7::8], C[:, B:, 7::8]], axis=2
    ).transpose(1, 0, 2).reshape(B, -1)                        # [64, 128]
    assert (bucket_min.max(axis=1) <= e_top[:, TOP_K - 1] + 1e-12).all(), (
        "top-k candidate extraction cannot prove exactness for this input"
    )

    S_all = A[:, :B, :].sum(axis=(0, 2)) + A[:, B:, :].sum(axis=(0, 2))  # [64]
    lam = labels_np
    c_star, r_star = np.divmod(lam, 8)
    h_star, g_star = np.divmod(r_star, 4)
    row = np.arange(B) + 64 * h_star
    S_pos = A[c_star, row, 2 * g_star] + A[c_star, row, 2 * g_star + 1]
    S_neg = S_all - S_pos

    loss_con = float(np.mean(np.log(e_top + S_neg[:, None]) - np.log(e_top)))

    logits = np.asarray(res.results[0]["lcT"]).astype(np.float64).T  # [64, 63]
    m = logits.max(axis=1, keepdims=True)
    lse = np.log(np.exp(logits - m).sum(axis=1, keepdims=True)) + m
    logp = logits - lse
    loss_cls = float(-np.mean(logp[np.arange(B), labels_np]))

    loss = 0.5 * loss_con + 0.5 * loss_cls
    return np.asarray(loss, dtype=np.float32)


# revision 15
# speedup vs baseline: 1.0207x; 1.0207x over previous
"""Distributed ContrastiveMoCoKnnBert loss kernel for 8 trn2 NeuronCores.

Math reduction (exact, not approximate):
  loss_con = -mean(log_softmax([pos | negs] / T)[:, 0]) over (B*TOP_K) rows.
  For row (b, j):  term = log(exp(p_bj/T) + sum_neg exp(n/T)) - p_bj/T
  where p_bj = j-th largest of cos_sim[b, :] (over ALL K columns) and the
  negative sum runs over columns whose queue label != labels[b].  The
  reference's top-NEG_MIN sort is irrelevant: softmax denominators are
  permutation invariant.  So the kernel only needs, per batch row:
    * top-25 values of cos_sim[b, :]        (monotonic under exp -> we
      extract top exp-values instead)
    * S_all[b] = sum_k exp(cos/T), S_pos[b] = sum_{label match} exp(cos/T)

Sharding: feature_queue is sorted by label on the host (1024 rows per
label, exactly balanced by construction), transposed, tiled, and split
along K into 8 shards of 8192 (= 8 labels x 1024) -- one per core:
  1. linerT = (tanh(q@Wd+bd)@Wo+bo).T unnormalized via transpose-free
     matmuls (host supplies qT in partition-major layout), plus its
     column norms (ones-matmul over squared entries)
  2. stream the fp8(e4m3, x256 host scale) fqT shard through TensorE:
     cos chunks [128, 512] f32 psum (partition = batch b + 64*half)
  3. ScalarE Exp with per-partition scale (2/(256*norm_b)) fused with
     per-512-chunk accum sums -> acc[128, 8]  (label-group sums = pairs
     of chunks, since each label spans 1024 sorted columns)
  4. one VectorE MAX8 per 512-col chunk -> top-8 bucket candidates
     cand[128, 64]; host PROVES completeness (bucket 8th-largest <=
     global 25th candidate) -- holds by a huge margin for random data
  5. classification head logits (replicated) -> lcT[63, 64]
Host merges: top-25 of the 1024 per-row candidates, S_neg = S_all-S_pos,
and assembles the scalar loss in f64.  All O(B*K*H) work is on device.
"""

import os

import numpy as np

import concourse.bass as bass
import concourse.bacc as bacc
import concourse.tile as tile
from concourse import mybir
from concourse.bass_utils import run_bass_kernel_spmd

B = 64
H = 768
K = 65536
L = 64            # NUM_LABELS
TOP_K = 25
T = 0.5
NCORES = 8
KSH = K // NCORES         # 8192 queue rows per core
NKC = H // 128            # 6 contraction chunks
CHUNK = 512               # psum-bank sized cos chunk
NJ = KSH // 2 // CHUNK    # 8 chunks per half
NH = 2                    # halves (partition packing: p = b + 64*h)
NCAND = 8                 # top-8 extracted per 512-col chunk

F32 = mybir.dt.float32
BF16 = mybir.dt.bfloat16
FP8 = mybir.dt.float8e4
FQ_FP8 = True             # stream feature queue as fp8e4m3 (x256 host scale)
FQ_DT = FP8 if FQ_FP8 else BF16
FQ_SCALE = 256.0 if FQ_FP8 else 1.0

_cache: dict = {}

last_exec_time_ns: int | None = None
last_results = None


def _ensure_ntff_hook():
    """Register the axon NTFF profiling hook if the image's antenv lacks
    the ``axon_hooks`` module (the hook impl itself ships in
    trn_agent_boot).  Also keep trace artifacts local instead of
    uploading to a share bucket."""
    import sys
    import types

    import concourse.bass_utils as bu

    bu.upload_artifacts = lambda tmpdir: tmpdir
    try:
        from antenv.axon_hooks import get_axon_ntff_profile_hook  # noqa: F401
        return
    except ImportError:
        pass
    try:
        from trn_agent_boot.trn_boot import _ntff_profile_via_ctypes
    except ImportError:
        return
    mod = types.ModuleType("antenv.axon_hooks")
    _hook = [None]
    mod.set_axon_ntff_profile_hook = lambda h: _hook.__setitem__(0, h)
    mod.get_axon_ntff_profile_hook = lambda: _hook[0]
    sys.modules["antenv.axon_hooks"] = mod
    import antenv

    antenv.axon_hooks = mod
    try:
        mod.set_axon_ntff_profile_hook(
            _ntff_profile_via_ctypes("/opt/axon/libaxon_pjrt.so")
        )
    except Exception:
        mod.set_axon_ntff_profile_hook(None)


def _build_nc():
    nc = bacc.Bacc(
        "TRN2",
        target_bir_lowering=False,
        debug=False,
        enable_asserts=False,
        num_devices=NCORES,
    )

    qT = nc.dram_tensor("qT", [128, NKC, B], BF16, kind="ExternalInput")
    wd = nc.dram_tensor("wd", [128, NKC, H], BF16, kind="ExternalInput")
    wo = nc.dram_tensor("wo", [128, NKC, H], BF16, kind="ExternalInput")
    wc1 = nc.dram_tensor("wc1", [128, NKC, H], BF16, kind="ExternalInput")
    wc2 = nc.dram_tensor("wc2", [128, NKC, L - 1], BF16, kind="ExternalInput")
    bdt = nc.dram_tensor("bd", [128, NKC], F32, kind="ExternalInput")
    bot = nc.dram_tensor("bo", [128, NKC], F32, kind="ExternalInput")
    bc1t = nc.dram_tensor("bc1", [128, NKC], F32, kind="ExternalInput")
    bc2t = nc.dram_tensor("bc2", [L - 1, 1], F32, kind="ExternalInput")
    fqt = nc.dram_tensor("fqt", [NJ, NKC, 128, NH * CHUNK], FQ_DT, kind="ExternalInput")

    cand_o = nc.dram_tensor("cand", [128, NJ * NCAND], BF16, kind="ExternalOutput")
    acc_o = nc.dram_tensor("acc", [128, NJ], F32, kind="ExternalOutput")
    lc_o = nc.dram_tensor("lcT", [L - 1, B], F32, kind="ExternalOutput")

    AF = mybir.ActivationFunctionType

    with tile.TileContext(nc) as tc:
        with (
            tc.tile_pool(name="weights", bufs=1) as wpool,
            tc.tile_pool(name="work", bufs=1) as spool,
            tc.tile_pool(name="fqstream", bufs=36) as fqpool,
            tc.tile_pool(name="cospsum", bufs=6, space="PSUM") as pspool,
            tc.tile_pool(name="headpsum", bufs=2, space="PSUM") as hpool,
        ):
            # ---- resident params --------------------------------------
            wd_sb = wpool.tile([128, NKC, H], BF16)
            wo_sb = wpool.tile([128, NKC, H], BF16)
            wc1_sb = wpool.tile([128, NKC, H], BF16)
            wc2_sb = wpool.tile([128, NKC, L - 1], BF16)
            qt_sb = wpool.tile([128, NKC, B], BF16)
            bd_sb = wpool.tile([128, NKC], F32)
            bo_sb = wpool.tile([128, NKC], F32)
            bc1_sb = wpool.tile([128, NKC], F32)
            bc2_sb = wpool.tile([L - 1, 1], F32)
            ones_sb = wpool.tile([128, 1], F32)

            # critical path (contrastive head) on the HWDGE queue, ahead
            # of the fq stream; cls-head params via SWDGE in parallel
            nc.sync.dma_start(qt_sb[:], qT.ap())
            nc.sync.dma_start(bd_sb[:], bdt.ap())
            nc.sync.dma_start(bo_sb[:], bot.ap())
            for kc in range(NKC):
                nc.sync.dma_start(wd_sb[:, kc, :], wd.ap()[:, kc, :])
            for kc in range(NKC):
                nc.sync.dma_start(wo_sb[:, kc, :], wo.ap()[:, kc, :])
            nc.gpsimd.dma_start(wc1_sb[:], wc1.ap())
            nc.gpsimd.dma_start(wc2_sb[:], wc2.ap())
            nc.gpsimd.dma_start(bc1_sb[:], bc1t.ap())
            nc.gpsimd.dma_start(bc2_sb[:], bc2t.ap())
            nc.vector.memset(ones_sb[:], 1.0)

            # ---- contrastive head: linerT (unnormalized) + norms ------
            h1_sb = spool.tile([128, NKC, B], BF16)
            pre2_sb = spool.tile([128, NKC, B], F32)
            pre2b_sb = spool.tile([128, NKC, B], BF16)
            sq_sb = spool.tile([128, NKC, B], F32)

            for mc in range(NKC):
                ps = hpool.tile([128, B], F32, tag="headps")
                for kc in range(NKC):
                    nc.tensor.matmul(
                        ps[:],
                        wd_sb[:, kc, mc * 128:(mc + 1) * 128],
                        qt_sb[:, kc, :],
                        start=(kc == 0),
                        stop=(kc == NKC - 1),
                    )
                nc.scalar.activation(
                    h1_sb[:, mc, :], ps[:], AF.Tanh, bias=bd_sb[:, mc:mc + 1]
                )

            for mc in range(NKC):
                ps = hpool.tile([128, B], F32, tag="headps")
                for kc in range(NKC):
                    nc.tensor.matmul(
                        ps[:],
                        wo_sb[:, kc, mc * 128:(mc + 1) * 128],
                        h1_sb[:, kc, :],
                        start=(kc == 0),
                        stop=(kc == NKC - 1),
                    )
                nc.vector.tensor_scalar_add(pre2_sb[:, mc, :], ps[:], bo_sb[:, mc:mc + 1])
                nc.vector.tensor_copy(pre2b_sb[:, mc, :], pre2_sb[:, mc, :])
                nc.vector.tensor_mul(sq_sb[:, mc, :], pre2_sb[:, mc, :], pre2_sb[:, mc, :])

            # column norms of pre2 (= row norms of liner_q), replicated to
            # both partition halves: norms2[p] for p = b and p = b + 64
            ps_n = hpool.tile([128, 1], F32, tag="headps")
            for hh in range(NH):
                for kc in range(NKC):
                    nc.tensor.matmul(
                        ps_n[hh * 64:(hh + 1) * 64, :],
                        sq_sb[:, kc, :],
                        ones_sb[:],
                        start=(kc == 0),
                        stop=(kc == NKC - 1),
                    )
            norm_sb = spool.tile([128, 1], F32)
            rcp_sb = spool.tile([128, 1], F32)
            scol_sb = spool.tile([128, 1], F32)
            nc.scalar.activation(norm_sb[:], ps_n[:], AF.Sqrt)
            nc.vector.reciprocal(rcp_sb[:], norm_sb[:])
            nc.vector.tensor_scalar_mul(scol_sb[:], rcp_sb[:], 1.0 / (T * FQ_SCALE))

            # ---- classification head (replicated, tiny) ---------------
            # emitted between head and stream: PE runs it while the cos
            # stream waits on fq DMA, keeping it off the critical tail
            h1c_sb = spool.tile([128, NKC, B], BF16)
            for mc in range(NKC):
                ps = hpool.tile([128, B], F32, tag="headps")
                for kc in range(NKC):
                    nc.tensor.matmul(
                        ps[:],
                        wc1_sb[:, kc, mc * 128:(mc + 1) * 128],
                        qt_sb[:, kc, :],
                        start=(kc == 0),
                        stop=(kc == NKC - 1),
                    )
                nc.scalar.activation(
                    h1c_sb[:, mc, :], ps[:], AF.Tanh, bias=bc1_sb[:, mc:mc + 1]
                )
            ps_l = hpool.tile([L - 1, B], F32, tag="headps")
            for kc in range(NKC):
                nc.tensor.matmul(
                    ps_l[:],
                    wc2_sb[:, kc, :],
                    h1c_sb[:, kc, :],
                    start=(kc == 0),
                    stop=(kc == NKC - 1),
                )
            lc_sb = spool.tile([L - 1, B], F32)
            nc.vector.tensor_scalar_add(lc_sb[:], ps_l[:], bc2_sb[:])
            nc.sync.dma_start(lc_o.ap(), lc_sb[:])

            # ---- main stream: cos chunks -> exp(+sums) -> topk --------
            acc_sb = spool.tile([128, NJ], F32)
            cand_sb = spool.tile([128, NJ * NCAND], BF16)

            for j in range(NJ):
                ps_c = pspool.tile([128, CHUNK], F32, tag="cos")
                fts = []
                for kc in range(NKC):
                    ft = fqpool.tile([128, NH * CHUNK], FQ_DT, tag="fq")
                    nc.sync.dma_start(ft[:], fqt.ap()[j, kc])
                    fts.append(ft)
                for hh in range(NH):
                    for kc in range(NKC):
                        nc.tensor.matmul(
                            ps_c[hh * 64:(hh + 1) * 64, :],
                            pre2b_sb[:, kc, :],
                            fts[kc][:, hh * CHUNK:(hh + 1) * CHUNK],
                            start=(kc == 0),
                            stop=(kc == NKC - 1),
                        )
                exp_t = fqpool.tile([128, CHUNK], BF16, tag="exp")
                nc.scalar.activation(
                    exp_t[:],
                    ps_c[:],
                    AF.Exp,
                    scale=scol_sb[:],
                    accum_out=acc_sb[:, j:j + 1],
                )
                nc.vector.max(cand_sb[:, j * NCAND:(j + 1) * NCAND], exp_t[:])

            nc.sync.dma_start(cand_o.ap(), cand_sb[:])
            nc.sync.dma_start(acc_o.ap(), acc_sb[:])

    nc.compile()
    return nc


def _get_nc():
    if "nc" not in _cache:
        _cache["nc"] = _build_nc()
    return _cache["nc"]


def _prep_inputs(q, label_queue, feature_queue, Wd, bd, Wo, bo, Wc1, bc1, Wc2, bc2):
    """Host-side shard/layout prep.  Returns per-core input maps."""
    lq = np.asarray(label_queue).astype(np.int64)
    counts = np.bincount(lq, minlength=L)
    assert counts.shape[0] == L and np.all(counts == K // L), (
        "kernel assumes an exactly balanced label queue"
    )
    perm = np.argsort(lq, kind="stable")
    fq_sorted = np.asarray(feature_queue, dtype=np.float32)[perm]  # [K, H]

    bf16 = mybir.dt.np(BF16)

    def pk(w, cols):  # [H, cols] -> partition-major [128, NKC, cols]
        return np.ascontiguousarray(
            np.asarray(w, np.float32).reshape(NKC, 128, cols).transpose(1, 0, 2)
        ).astype(bf16)

    common = {
        "qT": pk(np.asarray(q, np.float32).T, B),
        "wd": pk(Wd, H),
        "wo": pk(Wo, H),
        "wc1": pk(Wc1, H),
        "wc2": pk(Wc2, L - 1),
        "bd": np.ascontiguousarray(np.asarray(bd, np.float32).reshape(NKC, 128).T),
        "bo": np.ascontiguousarray(np.asarray(bo, np.float32).reshape(NKC, 128).T),
        "bc1": np.ascontiguousarray(np.asarray(bc1, np.float32).reshape(NKC, 128).T),
        "bc2": np.ascontiguousarray(np.asarray(bc2, np.float32).reshape(L - 1, 1)),
    }
    fq_dt = mybir.dt.np(FQ_DT)
    in_maps = []
    for c in range(NCORES):
        shard = fq_sorted[c * KSH:(c + 1) * KSH]          # [8192, H]
        fqT = np.ascontiguousarray(shard.T)               # [H, 8192]
        tiles = np.ascontiguousarray(
            (fqT * FQ_SCALE).reshape(NKC, 128, NH, NJ, CHUNK)
            .transpose(3, 0, 1, 2, 4)
            .reshape(NJ, NKC, 128, NH * CHUNK).astype(fq_dt)
        )
        in_maps.append({**common, "fqt": tiles})
    return in_maps


def kernel(
    q,
    labels,
    label_queue,
    feature_queue,
    Wd,
    bd,
    Wo,
    bo,
    Wc1,
    bc1,
    Wc2,
    bc2,
):
    global last_exec_time_ns, last_results
    nc = _get_nc()
    in_maps = _prep_inputs(
        q, label_queue, feature_queue, Wd, bd, Wo, bo, Wc1, bc1, Wc2, bc2
    )

    trace = os.environ.get("BASS_KERNEL_TRACE", "0") == "1"
    if trace:
        _ensure_ntff_hook()
    try:
        res = run_bass_kernel_spmd(
            nc,
            in_maps,
            core_ids=list(range(NCORES)),
            trace=trace,
            trace_cores=[0] if trace else None,
        )
    except Exception:
        if not trace:
            raise
        res = run_bass_kernel_spmd(nc, in_maps, core_ids=list(range(NCORES)))
    last_exec_time_ns = res.exec_time_ns
    last_results = res

    labels_np = np.asarray(labels).astype(np.int64)

    # ---- tiny host-side merge (the "gather + reduce" step) -----------
    C = np.stack([np.asarray(r["cand"]) for r in res.results]).astype(np.float64)
    A = np.stack([np.asarray(r["acc"]) for r in res.results]).astype(np.float64)

    # per-row candidate pool: cores x halves x (8 chunks * top-8)
    cand = np.concatenate([C[:, :B, :], C[:, B:, :]], axis=2)  # [8, 64, 128]
    cand = cand.transpose(1, 0, 2).reshape(B, -1)              # [64, 1024]
    e_top = np.sort(cand, axis=1)[:, ::-1][:, :TOP_K]          # exp(p/T) desc
    # Exactness proof: every unextracted value in a 512-col bucket is
    # <= that bucket's 8th-largest (MAX8 output is sorted desc).  If all
    # bucket minima are <= the global 25th candidate, the top-25 value
    # set is provably complete.
    bucket_min = np.concatenate(
        [C[:, :B, 7::8], C[:, B:, 7::8]], axis=2
    ).transpose(1, 0, 2).reshape(B, -1)                        # [64, 128]
    assert (bucket_min.max(axis=1) <= e_top[:, TOP_K - 1] + 1e-12).all(), (
        "top-k candidate extraction cannot prove exactness for this input"
    )

    S_all = A[:, :B, :].sum(axis=(0, 2)) + A[:, B:, :].sum(axis=(0, 2))  # [64]
    lam = labels_np
    c_star, r_star = np.divmod(lam, 8)
    h_star, g_star = np.divmod(r_star, 4)
    row = np.arange(B) + 64 * h_star
    S_pos = A[c_star, row, 2 * g_star] + A[c_star, row, 2 * g_star + 1]
    S_neg = S_all - S_pos

    loss_con = float(np.mean(np.log(e_top + S_neg[:, None]) - np.log(e_top)))

    logits = np.asarray(res.results[0]["lcT"]).astype(np.float64).T  # [64, 63]
    m = logits.max(axis=1, keepdims=True)
    lse = np.log(np.exp(logits - m).sum(axis=1, keepdims=True)) + m
    logp = logits - lse
    loss_cls = float(-np.mean(logp[np.arange(B), labels_np]))

    loss = 0.5 * loss_con + 0.5 * loss_cls
    return np.asarray(loss, dtype=np.float32)
